# revision 2
# baseline (speedup 1.0000x reference)
"""RNN-T joint network kernel for 8 Trainium2 NeuronCores.

out[b,t,u,:] = W2 @ tanh(W1e @ enc[b,t] + W1d @ dec[b,u] + b1) + b2

Shapes: B=4, T=200, U=100, D=512, H=1024, O=512 (fp32 in/out).
Sharding: T split 8 ways (25 t's per core); dec + weights replicated.

All matmul/elementwise traffic is bf16 (host pre-casts inputs): DMA bytes
halve, LDWEIGHTS gets FWL (2 elem/cycle), and the broadcast builds hit the
DVE 2x packed-16-bit mode. PSUM accumulation stays fp32; measured rel_fro
vs the fp32 reference is ~4e-3 (gate is 2e-2).

Per-core device program:
  Phase 1: per hk (8 chunks of 128 h): enc_h = W1e @ encT (4 dk matmuls)
           -> ACT writes e_dup[h, 2*pair] = enc_h + b1, duplicated into
           adjacent bf16 pairs; dec_h = W1d @ decT -> ACT copy dech bf16.
           (evacuations on ACT so DVE is free for early builds.)
  Phase 2: per chunk (b, up to 5 t's) = up to 500 rows:
           8 DVE adds s[k,t,u] = dech (+) e_dup with all innermost APs
           step-1 over bf16 pairs (both operands 2-packed per 32b read),
           one ACT tanh over [128, 8*rows], then per oc (4): 8 accumulating
           matmuls -> psum [128, rows], bias-add evacuation split ACT/DVE,
           DMA out.
"""

from contextlib import ExitStack

import numpy as np
import ml_dtypes

import concourse.bacc as bacc
import concourse.bass as bass
import concourse.mybir as mybir
import concourse.tile as tile
from concourse.bass_utils import run_bass_kernel_spmd

F32 = mybir.dt.float32
BF16 = mybir.dt.bfloat16
BF = ml_dtypes.bfloat16

B, T, U, D, H, O = 4, 200, 100, 512, 1024, 512
NCORES = 8
TLOC = T // NCORES            # 25 t's per core
PAIRS = B * TLOC              # 100 (b,t) pairs per core
TCH = 5                       # t's per inner chunk
CHROWS = TCH * U              # 500 rows per chunk
ROWS = PAIRS * U              # 10000 output rows per core
DK = D // 128                 # 4 contraction chunks for phase 1
HK = H // 128                 # 8 h chunks
BU = B * U                    # 400

_CACHE = {}


def _w1sl(ws, hk, dk):
    # host lays w1 out as [half][dk][hk%4][128] so each 2KB half is one
    # contiguous DMA (first matmul can start after half 0 lands)
    base = ((hk // 4) * DK + dk) * 512 + (hk % 4) * 128
    return ws[:, base:base + 128]


def _build():
    nc = bacc.Bacc("TRN2", target_bir_lowering=False, debug=False,
                   num_devices=NCORES)
    encT = nc.dram_tensor("encT", [128, DK * PAIRS], BF16, kind="ExternalInput")
    decT = nc.dram_tensor("decT", [128, DK * BU], BF16, kind="ExternalInput")
    w1eT = nc.dram_tensor("w1eT", [128, DK * H], BF16, kind="ExternalInput")
    w1dT = nc.dram_tensor("w1dT", [128, DK * H], BF16, kind="ExternalInput")
    w2T = nc.dram_tensor("w2T", [128, HK * O], BF16, kind="ExternalInput")
    b1r = nc.dram_tensor("b1r", [128, HK], F32, kind="ExternalInput")
    b2c = nc.dram_tensor("b2c", [128, O // 128], F32, kind="ExternalInput")
    out = nc.dram_tensor("out", [O, ROWS], F32, kind="ExternalOutput")

    HALF = DK * H // 2
    with tile.TileContext(nc) as tc, ExitStack() as ctx:
        consts = ctx.enter_context(tc.tile_pool(name="consts", bufs=1))
        spool = ctx.enter_context(tc.tile_pool(name="spool", bufs=4))
        opool = ctx.enter_context(tc.tile_pool(name="opool", bufs=8))
        psB = ctx.enter_context(tc.tile_pool(name="psB", bufs=8, space="PSUM"))

        w1e_s = consts.tile([128, DK * H], BF16)
        w1d_s = consts.tile([128, DK * H], BF16)
        w2_s = consts.tile([128, HK * O], BF16)
        encT_s = consts.tile([128, DK * PAIRS], BF16)
        decT_s = consts.tile([128, DK * BU], BF16)
        b1_s = consts.tile([128, HK], F32)
        b2c_s = consts.tile([128, O // 128], F32)
        # phase-1 results, bf16: e_dup holds each value twice (pair-packed
        # so phase-2 builds read it with innermost step 1)
        e_dup = consts.tile([128, HK * 2 * PAIRS], BF16)
        dech = consts.tile([128, HK * BU], BF16)

        # ---- input DMAs, ordered for earliest first matmul ----
        # sync ring: w1e halves, then w2; scalar ring: enc, b1, dec, w1d
        # halves, b2
        nc.sync.dma_start(w1e_s[:, :HALF], w1eT[:, :HALF])
        nc.sync.dma_start(w1e_s[:, HALF:], w1eT[:, HALF:])
        nc.sync.dma_start(w2_s[:], w2T[:])
        nc.scalar.dma_start(encT_s[:], encT[:])
        nc.scalar.dma_start(b1_s[:], b1r[:])
        nc.scalar.dma_start(decT_s[:], decT[:])
        nc.scalar.dma_start(w1d_s[:, :HALF], w1dT[:, :HALF])
        nc.scalar.dma_start(w1d_s[:, HALF:], w1dT[:, HALF:])
        nc.scalar.dma_start(b2c_s[:], b2c[:])

        # ---- phase 1a: enc_h -> e_dup (+b1), per hk ----
        for hk in range(HK):
            pe = psB.tile([128, 512], F32, tag="psB", name="pe")
            pe = pe[:, :PAIRS]
            for dk in range(DK):
                nc.tensor.matmul(
                    pe[:], lhsT=_w1sl(w1e_s, hk, dk),
                    rhs=encT_s[:, dk * PAIRS:(dk + 1) * PAIRS],
                    start=(dk == 0), stop=(dk == DK - 1),
                )
            o2 = hk * 2 * PAIRS
            outap = e_dup[:, o2:o2 + 2 * PAIRS].rearrange("p (t a) -> p t a", a=2)
            inap = pe.rearrange("p (t a) -> p t a", a=1)
            bin_, bout = bass.broadcast_tensor_aps(inap, outap)
            nc.scalar.activation(bout, bin_,
                                 mybir.ActivationFunctionType.Identity,
                                 bias=b1_s[:, hk:hk + 1])

        # ---- phase 1b: dec_h -> dech, per hk ----
        for hk in range(HK):
            pd = psB.tile([128, 512], F32, tag="psB", name="pd")
            pd = pd[:, :BU]
            for dk in range(DK):
                nc.tensor.matmul(
                    pd[:], lhsT=_w1sl(w1d_s, hk, dk),
                    rhs=decT_s[:, dk * BU:(dk + 1) * BU],
                    start=(dk == 0), stop=(dk == DK - 1),
                )
            nc.scalar.activation(dech[:, hk * BU:(hk + 1) * BU], pd[:],
                                 mybir.ActivationFunctionType.Identity)

        # ---- phase 2 ----
        # small leading chunks shorten the first build+tanh fill
        chunks = []
        for b in range(B):
            sizes = [1, 4] + [TCH] * 4 if b == 0 else [TCH] * (TLOC // TCH)
            t0c = 0
            for tch in sizes:
                chunks.append((b, t0c, tch))
                t0c += tch
        U2 = U // 2
        for b, t0c, tch in chunks:
            rows_c = tch * U
            s_t = spool.tile([128, HK * CHROWS], BF16, tag="s")
            for k in range(HK):
                # all innermost APs step over adjacent bf16 pairs -> DVE
                # packs 2 per 32-bit port read (2x mode)
                in0 = dech[:, k * BU + b * U: k * BU + (b + 1) * U]
                in0 = in0.rearrange("p (o u a) -> p o u a", o=1, a=2)
                c0 = k * 2 * PAIRS + (b * TLOC + t0c) * 2
                in1 = e_dup[:, c0:c0 + 2 * tch].rearrange(
                    "p (t o a) -> p t o a", o=1, a=2)
                bc0, bc1 = bass.broadcast_tensor_aps(in0, in1)
                outap = s_t[:, k * CHROWS: k * CHROWS + rows_c].rearrange(
                    "p (t u a) -> p t u a", t=tch, a=2)
                nc.vector.tensor_tensor(outap, bc0, bc1, mybir.AluOpType.add)
            s_used = s_t[:].rearrange("p (k c) -> p k c", k=HK)[:, :, :rows_c]
            nc.scalar.activation(s_used, s_used,
                                 mybir.ActivationFunctionType.Tanh)
            row0 = b * (TLOC * U) + t0c * U
            for oc in range(O // 128):
                ps = psB.tile([128, 512], F32, tag="psB")
                for k in range(HK):
                    nc.tensor.matmul(
                        ps[:, :rows_c],
                        lhsT=w2_s[:, k * O + oc * 128: k * O + (oc + 1) * 128],
                        rhs=s_t[:, k * CHROWS: k * CHROWS + rows_c],
                        start=(k == 0), stop=(k == HK - 1),
                    )
                ot = opool.tile([128, CHROWS], F32, tag="ot")
                if oc < 2:
                    nc.scalar.activation(
                        ot[:, :rows_c], ps[:, :rows_c],
                        mybir.ActivationFunctionType.Identity,
                        bias=b2c_s[:, oc:oc + 1])
                else:
                    nc.vector.tensor_scalar_add(
                        ot[:, :rows_c], ps[:, :rows_c], b2c_s[:, oc:oc + 1])
                nc.sync.dma_start(
                    out[oc * 128:(oc + 1) * 128, row0:row0 + rows_c],
                    ot[:, :rows_c])
    nc.compile()
    return nc


def _chunk128(a):
    # [n*128, w] -> [128, n*w]: partition p holds row k*128+p of chunk k
    n = a.shape[0] // 128
    return np.ascontiguousarray(
        a.reshape(n, 128, a.shape[1]).transpose(1, 0, 2).reshape(128, -1))


def _w1_prep(w):
    # [128, DK*H] -> halves layout [half][dk][hk%4][128]
    base = _chunk128(np.ascontiguousarray(w.T))           # [128, DK*H]
    v = base.reshape(128, DK, 2, 4, 128).transpose(0, 2, 1, 3, 4)
    return np.ascontiguousarray(v.reshape(128, -1))


def kernel(enc_state, dec_state, W1, b1, W2, b2, _trace=False):
    enc_state = np.ascontiguousarray(enc_state, dtype=np.float32)
    dec_state = np.ascontiguousarray(dec_state, dtype=np.float32)
    W1 = np.asarray(W1, dtype=np.float32)
    b1 = np.asarray(b1, dtype=np.float32)
    W2 = np.asarray(W2, dtype=np.float32)
    b2 = np.asarray(b2, dtype=np.float32)

    if "nc" not in _CACHE:
        _CACHE["nc"] = _build()
    nc = _CACHE["nc"]

    decT = _chunk128(dec_state.reshape(B * U, D).T.astype(BF))
    w1eT = _w1_prep(W1[:, :D].astype(BF))
    w1dT = _w1_prep(W1[:, D:].astype(BF))
    w2T = _chunk128(W2.T.astype(BF))
    b1r = np.ascontiguousarray(b1.reshape(HK, 128).T)
    b2cm = np.ascontiguousarray(b2.reshape(O // 128, 128).T)

    in_maps = []
    for c in range(NCORES):
        enc_c = enc_state[:, c * TLOC:(c + 1) * TLOC, :].reshape(PAIRS, D)
        encT_c = _chunk128(enc_c.T.astype(BF))
        in_maps.append({
            "encT": encT_c, "decT": decT, "w1eT": w1eT, "w1dT": w1dT,
            "w2T": w2T, "b1r": b1r, "b2c": b2cm,
        })

    res = run_bass_kernel_spmd(nc, in_maps, list(range(NCORES)), trace=_trace)
    out = np.empty((B, T, U, O), dtype=np.float32)
    for c in range(NCORES):
        out[:, c * TLOC:(c + 1) * TLOC] = (
            res.results[c]["out"].T.reshape(B, TLOC, U, O))
    if _trace:
        kernel.last_results = res
    return out


# revision 12
# speedup vs baseline: 1.1983x; 1.1983x over previous
"""RNN-T joint network kernel for 8 Trainium2 NeuronCores.

out[b,t,u,:] = W2 @ tanh(W1e @ enc[b,t] + W1d @ dec[b,u] + b1) + b2

Shapes: B=4, T=200, U=100, D=512, H=1024, O=512 (fp32 in/out).
Sharding: T split 8 ways (25 t's per core); dec + weights replicated.

All matmul/elementwise traffic is bf16 (host pre-casts inputs): DMA bytes
halve, LDWEIGHTS gets FWL (2 elem/cycle), and the broadcast builds hit the
DVE 2x packed-16-bit mode. PSUM accumulation stays fp32; measured rel_fro
vs the fp32 reference is ~4e-3 (gate is 2e-2).

Per-core device program:
  Phase 1: per hk (8 chunks of 128 h): enc_h = W1e @ encT (4 dk matmuls)
           -> ACT writes e_dup[h, 2*pair] = enc_h + b1, duplicated into
           adjacent bf16 pairs; dec_h = W1d @ decT -> ACT copy dech bf16.
           (evacuations on ACT so DVE is free for early builds.)
  Phase 2: per chunk (b, up to 5 t's) = up to 500 rows:
           8 DVE adds s[k,t,u] = dech (+) e_dup with all innermost APs
           step-1 over bf16 pairs (both operands 2-packed per 32b read),
           one ACT tanh over [128, 8*rows], then per oc (4): 8 accumulating
           matmuls -> psum [128, rows], bias-add evacuation split ACT/DVE,
           DMA out.
"""

from contextlib import ExitStack

import numpy as np
import ml_dtypes

import concourse.bacc as bacc
import concourse.bass as bass
import concourse.mybir as mybir
import concourse.tile as tile
from concourse.bass_utils import run_bass_kernel_spmd

F32 = mybir.dt.float32
BF16 = mybir.dt.bfloat16
BF = ml_dtypes.bfloat16

B, T, U, D, H, O = 4, 200, 100, 512, 1024, 512
NCORES = 8
TLOC = T // NCORES            # 25 t's per core
PAIRS = B * TLOC              # 100 (b,t) pairs per core
TCH = 5                       # t's per inner chunk
CHROWS = TCH * U              # 500 rows per chunk
ROWS = PAIRS * U              # 10000 output rows per core
DK = D // 128                 # 4 contraction chunks for phase 1
HK = H // 128                 # 8 h chunks
BU = B * U                    # 400

_CACHE = {}


PADP = 128                    # encT dk-chunk stride (padded from 100 for
                              # 16B-aligned matmul rhs slices)
SSTR = 512                    # s-tile per-k stride (padded from 500; odd-k
                              # 1000B offsets would misalign the PE rhs
                              # stream and cost ~40ns per matmul)


def _w1sl(ws, hk, dk):
    # host lays w1 out as [quarter (hk//2)][dk][hk%2][128] so it loads as
    # four contiguous DMAs (first matmul can start after quarter 0 lands)
    base = ((hk // 2) * DK + dk) * 256 + (hk % 2) * 128
    return ws[:, base:base + 128]


def _build():
    nc = bacc.Bacc("TRN2", target_bir_lowering=False, debug=False,
                   num_devices=NCORES)
    encT = nc.dram_tensor("encT", [128, DK * PADP], BF16, kind="ExternalInput")
    decT = nc.dram_tensor("decT", [128, DK * BU], BF16, kind="ExternalInput")
    w1eT = nc.dram_tensor("w1eT", [128, DK * H], BF16, kind="ExternalInput")
    w1dT = nc.dram_tensor("w1dT", [128, DK * H], BF16, kind="ExternalInput")
    w2T = nc.dram_tensor("w2T", [128, HK * O], BF16, kind="ExternalInput")
    b1r = nc.dram_tensor("b1r", [128, HK], F32, kind="ExternalInput")
    b2c = nc.dram_tensor("b2c", [128, O // 128], F32, kind="ExternalInput")
    out = nc.dram_tensor("out", [O, ROWS], F32, kind="ExternalOutput")

    QTR = DK * H // 4
    with tile.TileContext(nc) as tc, ExitStack() as ctx:
        consts = ctx.enter_context(tc.tile_pool(name="consts", bufs=1))
        spool = ctx.enter_context(tc.tile_pool(name="spool", bufs=4))
        opool = ctx.enter_context(tc.tile_pool(name="opool", bufs=8))
        psB = ctx.enter_context(tc.tile_pool(name="psB", bufs=8, space="PSUM"))

        w1e_s = consts.tile([128, DK * H], BF16)
        w1d_s = consts.tile([128, DK * H], BF16)
        w2_s = consts.tile([128, HK * O], BF16)
        encT_s = consts.tile([128, DK * PADP], BF16)
        decT_s = consts.tile([128, DK * BU], BF16)
        b1_s = consts.tile([128, HK], F32)
        b2c_s = consts.tile([128, O // 128], F32)
        # phase-1 results, bf16: e_dup holds each value twice (pair-packed
        # so phase-2 builds read it with innermost step 1)
        e_dup = consts.tile([128, HK * 2 * PAIRS], BF16)
        dech = consts.tile([128, HK * BU], BF16)

        # ---- input DMAs, ordered for earliest first matmul ----
        # sync ring: w1e q0, enc, w1e q1-3, w2; scalar ring: b1, dec,
        # w1d quarters, b2
        nc.sync.dma_start(w1e_s[:, :QTR], w1eT[:, :QTR])
        nc.sync.dma_start(encT_s[:], encT[:])
        for q in range(1, 4):
            nc.sync.dma_start(w1e_s[:, q * QTR:(q + 1) * QTR],
                              w1eT[:, q * QTR:(q + 1) * QTR])
        nc.sync.dma_start(w2_s[:], w2T[:])
        nc.scalar.dma_start(b1_s[:], b1r[:])
        nc.scalar.dma_start(decT_s[:], decT[:])
        for q in range(4):
            nc.scalar.dma_start(w1d_s[:, q * QTR:(q + 1) * QTR],
                                w1dT[:, q * QTR:(q + 1) * QTR])
        nc.scalar.dma_start(b2c_s[:], b2c[:])

        # ---- phase 1a: enc_h -> e_dup (+b1), per hk ----
        for hk in range(HK):
            pe = psB.tile([128, 512], F32, tag="psB", name="pe")
            pe = pe[:, :PAIRS]
            for dk in range(DK):
                nc.tensor.matmul(
                    pe[:], lhsT=_w1sl(w1e_s, hk, dk),
                    rhs=encT_s[:, dk * PADP: dk * PADP + PAIRS],
                    start=(dk == 0), stop=(dk == DK - 1),
                )
            o2 = hk * 2 * PAIRS
            outap = e_dup[:, o2:o2 + 2 * PAIRS].rearrange("p (t a) -> p t a", a=2)
            inap = pe.rearrange("p (t a) -> p t a", a=1)
            bin_, bout = bass.broadcast_tensor_aps(inap, outap)
            nc.scalar.activation(bout, bin_,
                                 mybir.ActivationFunctionType.Identity,
                                 bias=b1_s[:, hk:hk + 1])

        # ---- phase 1b: dec_h -> dech, per hk ----
        for hk in range(HK):
            pd = psB.tile([128, 512], F32, tag="psB", name="pd")
            pd = pd[:, :BU]
            for dk in range(DK):
                nc.tensor.matmul(
                    pd[:], lhsT=_w1sl(w1d_s, hk, dk),
                    rhs=decT_s[:, dk * BU:(dk + 1) * BU],
                    start=(dk == 0), stop=(dk == DK - 1),
                )
            nc.scalar.activation(dech[:, hk * BU:(hk + 1) * BU], pd[:],
                                 mybir.ActivationFunctionType.Identity)

        # ---- phase 2 ----
        # small leading chunks shorten the first build+tanh fill
        chunks = []
        for b in range(B):
            sizes = [1, 4] + [TCH] * 4 if b == 0 else [TCH] * (TLOC // TCH)
            t0c = 0
            for tch in sizes:
                chunks.append((b, t0c, tch))
                t0c += tch
        U2 = U // 2
        first = True
        for b, t0c, tch in chunks:
            rows_c = tch * U
            s_t = spool.tile([128, HK * SSTR], BF16, tag="s")
            for k in range(HK):
                # all innermost APs step over adjacent bf16 pairs -> DVE
                # packs 2 per 32-bit port read (2x mode)
                in0 = dech[:, k * BU + b * U: k * BU + (b + 1) * U]
                in0 = in0.rearrange("p (o u a) -> p o u a", o=1, a=2)
                c0 = k * 2 * PAIRS + (b * TLOC + t0c) * 2
                in1 = e_dup[:, c0:c0 + 2 * tch].rearrange(
                    "p (t o a) -> p t o a", o=1, a=2)
                bc0, bc1 = bass.broadcast_tensor_aps(in0, in1)
                outap = s_t[:, k * SSTR: k * SSTR + rows_c].rearrange(
                    "p (t u a) -> p t u a", t=tch, a=2)
                nc.vector.tensor_tensor(outap, bc0, bc1, mybir.AluOpType.add)
                if first:
                    # per-k tanh on the very first chunk so its matmuls can
                    # trail the phase-1 tail with minimal PE idle
                    sk = s_t[:, k * SSTR: k * SSTR + rows_c]
                    nc.scalar.activation(sk, sk,
                                         mybir.ActivationFunctionType.Tanh)
            if not first:
                s_used = s_t[:].rearrange(
                    "p (k c) -> p k c", k=HK)[:, :, :rows_c]
                nc.scalar.activation(s_used, s_used,
                                     mybir.ActivationFunctionType.Tanh)
            first = False
            row0 = b * (TLOC * U) + t0c * U
            for oc in range(O // 128):
                ps = psB.tile([128, 512], F32, tag="psB")
                for k in range(HK):
                    nc.tensor.matmul(
                        ps[:, :rows_c],
                        lhsT=w2_s[:, k * O + oc * 128: k * O + (oc + 1) * 128],
                        rhs=s_t[:, k * SSTR: k * SSTR + rows_c],
                        start=(k == 0), stop=(k == HK - 1),
                    )
                ot = opool.tile([128, CHROWS], F32, tag="ot")
                if oc < 2:
                    nc.scalar.activation(
                        ot[:, :rows_c], ps[:, :rows_c],
                        mybir.ActivationFunctionType.Identity,
                        bias=b2c_s[:, oc:oc + 1])
                else:
                    nc.vector.tensor_scalar_add(
                        ot[:, :rows_c], ps[:, :rows_c], b2c_s[:, oc:oc + 1])
                nc.sync.dma_start(
                    out[oc * 128:(oc + 1) * 128, row0:row0 + rows_c],
                    ot[:, :rows_c])
    nc.compile()
    return nc


def _chunk128(a):
    # [n*128, w] -> [128, n*w]: partition p holds row k*128+p of chunk k
    n = a.shape[0] // 128
    return np.ascontiguousarray(
        a.reshape(n, 128, a.shape[1]).transpose(1, 0, 2).reshape(128, -1))


def _w1_prep(w):
    # [128, DK*H] -> quarters layout [hk//2][dk][hk%2][128]
    base = _chunk128(np.ascontiguousarray(w.T))           # [128, DK*H]
    v = base.reshape(128, DK, 4, 2, 128).transpose(0, 2, 1, 3, 4)
    return np.ascontiguousarray(v.reshape(128, -1))


def kernel(enc_state, dec_state, W1, b1, W2, b2, _trace=False):
    enc_state = np.ascontiguousarray(enc_state, dtype=np.float32)
    dec_state = np.ascontiguousarray(dec_state, dtype=np.float32)
    W1 = np.asarray(W1, dtype=np.float32)
    b1 = np.asarray(b1, dtype=np.float32)
    W2 = np.asarray(W2, dtype=np.float32)
    b2 = np.asarray(b2, dtype=np.float32)

    if "nc" not in _CACHE:
        _CACHE["nc"] = _build()
    nc = _CACHE["nc"]

    decT = _chunk128(dec_state.reshape(B * U, D).T.astype(BF))
    w1eT = _w1_prep(W1[:, :D].astype(BF))
    w1dT = _w1_prep(W1[:, D:].astype(BF))
    w2T = _chunk128(W2.T.astype(BF))
    b1r = np.ascontiguousarray(b1.reshape(HK, 128).T)
    b2cm = np.ascontiguousarray(b2.reshape(O // 128, 128).T)

    in_maps = []
    for c in range(NCORES):
        enc_c = enc_state[:, c * TLOC:(c + 1) * TLOC, :].reshape(PAIRS, D)
        encT_c = _chunk128(enc_c.T.astype(BF))            # [128, DK*PAIRS]
        # pad dk chunks to 128 cols so matmul rhs slices are 16B-aligned
        encT_c = np.ascontiguousarray(
            np.pad(encT_c.reshape(128, DK, PAIRS),
                   ((0, 0), (0, 0), (0, PADP - PAIRS))).reshape(128, -1))
        in_maps.append({
            "encT": encT_c, "decT": decT, "w1eT": w1eT, "w1dT": w1dT,
            "w2T": w2T, "b1r": b1r, "b2c": b2cm,
        })

    res = run_bass_kernel_spmd(nc, in_maps, list(range(NCORES)), trace=_trace)
    out = np.empty((B, T, U, O), dtype=np.float32)
    for c in range(NCORES):
        out[:, c * TLOC:(c + 1) * TLOC] = (
            res.results[c]["out"].T.reshape(B, TLOC, U, O))
    if _trace:
        kernel.last_results = res
    return out
